# revision 1
# baseline (speedup 1.0000x reference)
"""Trainium2 Bass kernel for the AggregPolicy GNN message-passing model.

Math: the reference network is, per batch row x (18 features):
    s0 = E @ x_feats + d          (state s = [h_j[0..6] (7*4), h_m (4)] = 32 dims)
    s  = tanh(M @ s + c)          x 7   (chain-graph message passing folded into
                                         one dense 32x32 matrix M)
    out = F @ s + bact            (7 outputs)
The first iteration is fused with the init affine: s1 = tanh(G @ x + g) with
G = M @ E, g = M @ d + c.  All matrices are precomputed on the host from the
tiny model weights.

Layout on chip (per NeuronCore, pure data parallel over 8 cores):
  - 4 batch "chunks" x 32 state dims are stacked on the 128 SBUF partitions
    (block-diagonal G/M/F), batch runs along the free dimension.
  - batch rows are interleaved mod 4 across chunks, so each SBUF partition of
    an input tile holds 4 *consecutive* DRAM rows (288 B contiguous reads) and
    each partition of an output tile holds 4 consecutive rows of y (112 B
    contiguous writes).
  - Per 128-column tile: PE transposes x [128,72] -> [72,128] (features to
    partitions), then one matmul per iteration per 512-col slice; ScalarE
    applies tanh+bias straight PSUM->SBUF; final per-tile matmul uses the
    state tile itself as the stationary operand to emit batch-major outputs.
"""

import os

os.environ.setdefault("MYCRO_LOCAL_CACHE", "1")

from contextlib import ExitStack

import numpy as np

import concourse.bacc as bacc
import concourse.tile as tile
from concourse import mybir
from concourse.bass_utils import run_bass_kernel_spmd

F32 = mybir.dt.float32
BF16 = mybir.dt.bfloat16
FP16 = mybir.dt.float16
# iteration/final matmuls in 16-bit (PE streams 1 col/cycle even cold, FWL
# weight loads); tanh still reads/writes via fp32 PSUM so only the matmul
# inputs are rounded. fp16 keeps 10 mantissa bits (values are all O(1), so
# no range risk) vs bf16's 7. K_SDT: fp16 | bf16 | f32r.
S_DT_KIND = os.environ.get("K_SDT", "fp16")
S_BF16 = S_DT_KIND in ("bf16", "fp16")  # 16-bit path enabled

N_CORES = 8
B_TOTAL = 2_000_000
R = B_TOTAL // N_CORES  # 250000 rows per core
NF = 18  # input features
NS = 32  # state dims
NO = 7  # outputs
CH = 4  # batch chunks stacked on partitions (4*32 = 128)
T_FULL = 128  # batch columns per tile (rows per tile = 4*T_FULL = 512)
BB_TILES = 12  # tiles per big-batch (ACT op free dim = 128*BB_TILES)

FULL_TILES = R // (CH * T_FULL)  # 488
TAIL_ROWS = R - FULL_TILES * CH * T_FULL  # 144
TAIL_T = TAIL_ROWS // CH  # 36
assert TAIL_T * CH == TAIL_ROWS


def build_host_constants(Wj, bj, Wm, bm, Wih_j, Whh_j, bih_j, bhh_j,
                         Wih_m, Whh_m, bih_m, bhh_m, Wact, bact):
    """Fold the model into (G_lhsT [72,128], M_lhsT [128,128], F_rhs [128,28],
    bias [128,3]) in the block-diagonal on-chip layouts."""
    H = 4
    M = np.zeros((NS, NS), np.float32)
    c = np.zeros((NS,), np.float32)
    for n in range(7):
        r = slice(4 * n, 4 * n + 4)
        if n == 0:
            M[r, 28:32] += Wih_j  # prev neighbor of node 0 is h_m
        else:
            M[r, 4 * (n - 1):4 * n] += Wih_j
        if n < 6:
            M[r, 4 * (n + 1):4 * (n + 2)] += Wih_j
        M[r, 4 * n:4 * n + 4] += Whh_j
        c[r] = bih_j + bhh_j
    M[28:32, 0:4] = Wih_m
    M[28:32, 28:32] = Whh_m
    c[28:32] = bih_m + bhh_m

    E = np.zeros((NS, NF), np.float32)
    d = np.zeros((NS,), np.float32)
    for n in range(7):
        for h in range(H):
            E[4 * n + h, 4 + n] = Wj[h, 0]
            E[4 * n + h, 11 + n] = Wj[h, 1]
        d[4 * n:4 * n + 4] = bj
    E[28:32, 0:4] = Wm
    d[28:32] = bm

    F = np.zeros((NO, NS), np.float32)
    for n in range(7):
        F[n, 4 * n:4 * n + 4] = Wact[0]

    G = (M @ E).astype(np.float32)
    g = (M @ d + c).astype(np.float32)

    # Block-diagonal device layouts.
    # G matmul: out[32c+o, col] = sum_f G[o,f] * xT[18c+f, col]
    G_lhsT = np.zeros((CH * NF, 128), np.float32)
    for cc in range(CH):
        G_lhsT[NF * cc:NF * (cc + 1), NS * cc:NS * (cc + 1)] = G.T
    # M matmul: out[32c+o, col] = sum_k M[o,k] * s[32c+k, col]
    M_lhsT = np.zeros((128, 128), np.float32)
    for cc in range(CH):
        M_lhsT[NS * cc:NS * (cc + 1), NS * cc:NS * (cc + 1)] = M.T
    # Final: out[t, 7c+j] = sum_{o} s7[32c+o, t] * F_rhs[32c+o, 7c+j]
    F_rhs = np.zeros((128, CH * NO), np.float32)
    for cc in range(CH):
        F_rhs[NS * cc:NS * (cc + 1), NO * cc:NO * (cc + 1)] = F.T
    # Per-partition bias vectors: col 0 = g (first iter), col 1 = c, col 2 = bact
    bias = np.zeros((128, 3), np.float32)
    bias[:, 0] = np.tile(g, CH)
    bias[:, 1] = np.tile(c, CH)
    bias[:, 2] = float(bact[0])
    identity = np.eye(128, dtype=np.float32)
    return G_lhsT, M_lhsT, F_rhs, bias, identity


def build_tile_kernel(ctx, tc, x, y, gw, mw, fw, idm, bvec, rows, nrep=1):
    """Emit the Tile program. x:[rows,18], y:[rows,7] DRAM APs; consts in DRAM.

    Two independent big-batch streams (slot 0/1) are emitted with their
    per-iteration stages interleaved so the in-order ScalarE stream alternates
    tanh(A), tanh(B) back-to-back while the PE runs the other stream's matmuls.

    nrep > 1 wraps the whole body in a hardware For_i loop that recomputes the
    same outputs nrep times — used only for wall-clock benchmarking."""
    nc = tc.nc
    full_tiles = rows // (CH * T_FULL)
    tail_rows = rows - full_tiles * CH * T_FULL
    tail_t = tail_rows // CH
    assert tail_t * CH == tail_rows

    consts = ctx.enter_context(tc.tile_pool(name="consts", bufs=1))
    # per-stream SBUF pools
    xsb_p = [ctx.enter_context(tc.tile_pool(name=f"xsb{s}", bufs=2)) for s in (0, 1)]
    xf_p = [ctx.enter_context(tc.tile_pool(name=f"xf{s}", bufs=2)) for s in (0, 1)]
    s_p = [ctx.enter_context(tc.tile_pool(name=f"sp{s}", bufs=3)) for s in (0, 1)]
    osb_p = [ctx.enter_context(tc.tile_pool(name=f"osb{s}", bufs=2)) for s in (0, 1)]
    # PSUM: 1 shared xt buf (1 bank) + 1 shared outps buf (1 bank)
    # + one 3-bank pre per stream = 8 banks
    xt_pool = ctx.enter_context(tc.tile_pool(name="xt", bufs=1, space="PSUM"))
    ops_pool = ctx.enter_context(tc.tile_pool(name="ops", bufs=1, space="PSUM"))
    pre_p = [ctx.enter_context(tc.tile_pool(name=f"pre{s}", bufs=1, space="PSUM"))
             for s in (0, 1)]

    TANH = mybir.ActivationFunctionType.Tanh
    # benchmark-only ablation knobs (default = full correct kernel)
    N_TANH_ITERS = int(os.environ.get("K_ITERS", "7"))
    SKIP_F = bool(os.environ.get("K_SKIP_F"))
    SKIP_TR = bool(os.environ.get("K_SKIP_TR"))
    SKIP_DMA_IN = bool(os.environ.get("K_SKIP_DMA_IN"))
    SKIP_DMA_OUT = bool(os.environ.get("K_SKIP_DMA_OUT"))
    # float32r: same 4-byte storage, but the PE streams it at 1 cycle/col
    # (fp32 runs as 2 half-speed passes = 4 cycles/col). The BIR verifier
    # requires every producer feeding an f32r matmul to be typed f32r.
    F32R = mybir.dt.float32r
    SDT = {"bf16": BF16, "fp16": FP16}.get(S_DT_KIND, F32R)
    x = x.bitcast(F32R)
    idm = idm.bitcast(F32R)
    if not S_BF16:
        gw = gw.bitcast(F32R)
        mw = mw.bitcast(F32R)
        fw = fw.bitcast(F32R)

    g_sb = consts.tile([CH * NF, 128], SDT, name="g_sb")
    nc.sync.dma_start(g_sb[:], gw)
    m_sb = consts.tile([128, 128], SDT, name="m_sb")
    nc.sync.dma_start(m_sb[:], mw)
    f_sb = consts.tile([128, CH * NO], SDT, name="f_sb")
    nc.sync.dma_start(f_sb[:], fw)
    id_sb = consts.tile([128, 128], F32R, name="id_sb")
    nc.sync.dma_start(id_sb[:], idm)
    b_sb = consts.tile([128, 3], F32, name="b_sb")
    nc.sync.dma_start(b_sb[:], bvec)

    CAP = T_FULL * BB_TILES  # tile capacity (columns) of the per-stream bufs

    def load_x(sl, bb):
        row0, ntile, t_last = bb
        widths = [T_FULL] * (ntile - 1) + [t_last]
        nrows = CH * sum(widths)
        dense = t_last == T_FULL
        x_sb = xsb_p[sl].tile([128, NF * CH * BB_TILES], F32R,
                              name=f"x_sb{sl}", tag="x_sb")
        if SKIP_DMA_IN:
            # bench-only: tiny write so Tile sees the tile as allocated
            # (memset rejects f32r dtype at codegen, so write via f32 view)
            nc.vector.memset(x_sb[:, 0:2].bitcast(F32), 0.5)
        elif dense and os.environ.get("K_OLD_DMA"):
            srcx = x[row0:row0 + nrows, :].rearrange(
                "(k t c) f -> t k c f", k=ntile, t=T_FULL, c=CH)
            dstx = x_sb[:, 0:NF * CH * ntile].rearrange(
                "t (k c f) -> t k c f", k=ntile, c=CH, f=NF)
            nc.sync.dma_start(dstx, srcx)
        elif dense:
            # row mapping row0 + 48*t + 4*m + c: partition t holds 48
            # consecutive DRAM rows -> one 3456 B contiguous read per
            # partition (>=512 B avoids the sub-line 2x DMA penalty).
            rpp = CH * ntile  # rows per partition
            srcx = x[row0:row0 + nrows, :].rearrange(
                "(t r) f -> t r f", t=T_FULL, r=rpp)
            dstx = x_sb[:, 0:NF * CH * ntile].rearrange(
                "t (r f) -> t r f", r=rpp, f=NF)
            nc.sync.dma_start(dstx, srcx)
        else:
            for ti, w in enumerate(widths):
                r0 = row0 + CH * sum(widths[:ti])
                srcx = x[r0:r0 + CH * w, :].rearrange("(t c) f -> t c f", t=w, c=CH)
                dstx = x_sb[0:w, NF * CH * ti:NF * CH * (ti + 1)].rearrange(
                    "t (c f) -> t c f", c=CH, f=NF)
                nc.sync.dma_start(dstx, srcx)
        return x_sb

    def transpose_x(sl, bb, x_sb):
        row0, ntile, t_last = bb
        widths = [T_FULL] * (ntile - 1) + [t_last]
        xfeat = xf_p[sl].tile([CH * NF, CAP], SDT, name=f"xfeat{sl}", tag="xfeat")
        if SKIP_TR:
            return xfeat
        for xg0 in range(0, ntile, 4):
            gcnt = min(4, ntile - xg0)
            xt = xt_pool.tile([CH * NF, 512], F32R, name="xt", tag="xt")
            for i in range(gcnt):
                ti = xg0 + i
                w = widths[ti]
                nc.tensor.transpose(
                    out=xt[0:CH * NF, 128 * i:128 * i + w],
                    in_=x_sb[0:w, NF * CH * ti:NF * CH * (ti + 1)],
                    identity=id_sb[0:w, 0:w],
                )
            if t_last == T_FULL:
                nc.vector.tensor_copy(
                    xfeat[0:CH * NF, 128 * xg0:128 * xg0 + 128 * gcnt],
                    xt[0:CH * NF, 0:128 * gcnt])
            else:
                for i in range(gcnt):
                    ti = xg0 + i
                    w = widths[ti]
                    nc.vector.tensor_copy(
                        xfeat[0:CH * NF, 128 * ti:128 * ti + w],
                        xt[0:CH * NF, 128 * i:128 * i + w])
        return xfeat

    def mm_sliced(pre, lhsT, rhs_tile, rhs_parts, width):
        for q0 in range(0, width, 512):
            w = min(512, width - q0)
            nc.tensor.matmul(
                pre[:, q0:q0 + w],
                lhsT=lhsT,
                rhs=rhs_tile[0:rhs_parts, q0:q0 + w],
                start=True, stop=True)

    def emit_loads(bbs):
        st = {}
        for sl, bb in bbs:
            st[sl] = {"bb": bb}
            st[sl]["ncols"] = T_FULL * (bb[1] - 1) + bb[2]
            st[sl]["x_sb"] = load_x(sl, bb)
        return st

    def emit_tr(bbs, st):
        for sl, bb in bbs:
            st[sl]["xfeat"] = transpose_x(sl, bb, st[sl]["x_sb"])

    def emit_g(bbs, st):
        """G-matmul only: in the in-order PE stream this lands right after
        the previous body's M7, so G(A) overlaps tanh7(B) and tanh1 of this
        group starts with no ACT gap (the pre-tile WAR on tanh7 is the only
        wait). Transposes/copies must already be done (emit_tr mid-body)."""
        for sl, bb in bbs:
            pre = pre_p[sl].tile([128, CAP], F32, name=f"pre{sl}", tag="pre")
            st[sl]["pre"] = pre
            mm_sliced(pre, g_sb[:], st[sl]["xfeat"], CH * NF, st[sl]["ncols"])

    def emit_body(bbs, st, mid=None):
        """mid() is called after two extra iterations — PE/DVE idle time in
        the middle of the body — to emit the NEXT group's transposes+copies
        off the boundary critical path."""
        for sl, bb in bbs:
            s_t = s_p[sl].tile([128, CAP], SDT, name=f"s{sl}", tag="s")
            st[sl]["s"] = s_t
            nc.scalar.activation(s_t[:, 0:st[sl]["ncols"]],
                                 st[sl]["pre"][:, 0:st[sl]["ncols"]], TANH,
                                 bias=b_sb[:, 0:1], scale=1.0)
        # iterations 2..7
        mid_done = mid is None
        for it in range(N_TANH_ITERS - 1):
            if it == 2 and not mid_done:
                mid()
                mid_done = True
            for sl, bb in bbs:
                mm_sliced(st[sl]["pre"], m_sb[:], st[sl]["s"], 128,
                          st[sl]["ncols"])
            for sl, bb in bbs:
                nc.scalar.activation(st[sl]["s"][:, 0:st[sl]["ncols"]],
                                     st[sl]["pre"][:, 0:st[sl]["ncols"]], TANH,
                                     bias=b_sb[:, 1:2], scale=1.0)
        if not mid_done:
            mid()
    def emit_final(bbs, st):
        # final per-tile matmuls go to a separate PSUM bank so they stay off
        # the pre-tile WAR chain (next BB's G-matmul only waits on tanh7).
        for sl, bb in bbs:
            if SKIP_F:
                continue
            row0, ntile, t_last = bb
            widths = [T_FULL] * (ntile - 1) + [t_last]
            outps = ops_pool.tile([128, CH * NO * BB_TILES], F32,
                                  name="outps", tag="outps")
            st[sl]["outps"] = outps
            for ti, w in enumerate(widths):
                nc.tensor.matmul(
                    outps[0:w, CH * NO * ti:CH * NO * (ti + 1)],
                    lhsT=st[sl]["s"][:, 128 * ti:128 * ti + w],
                    rhs=f_sb[:],
                    start=True, stop=True)
        for sl, bb in bbs:
            if SKIP_F:
                continue
            row0, ntile, t_last = bb
            widths = [T_FULL] * (ntile - 1) + [t_last]
            dense = t_last == T_FULL
            out_sb = osb_p[sl].tile([128, CH * NO * BB_TILES], F32,
                                    name=f"out_sb{sl}", tag="out_sb")
            outps = st[sl]["outps"]
            if dense:
                nc.vector.tensor_scalar_add(
                    out_sb[:, 0:CH * NO * ntile],
                    outps[:, 0:CH * NO * ntile], b_sb[:, 2:3])
            else:
                for ti, w in enumerate(widths):
                    nc.vector.tensor_scalar_add(
                        out_sb[0:w, CH * NO * ti:CH * NO * (ti + 1)],
                        outps[0:w, CH * NO * ti:CH * NO * (ti + 1)],
                        b_sb[0:w, 2:3])
            nrows = CH * sum(widths)
            if SKIP_DMA_OUT:
                continue
            if dense and os.environ.get("K_OLD_DMA"):
                dsty = y[row0:row0 + nrows, :].rearrange(
                    "(k t c) j -> t k c j", k=ntile, t=T_FULL, c=CH)
                srcy = out_sb[:, 0:CH * NO * ntile].rearrange(
                    "t (k c j) -> t k c j", k=ntile, c=CH, j=NO)
                nc.sync.dma_start(dsty, srcy)
            elif dense:
                # same row mapping as the dense load: 48 consecutive rows per
                # partition -> 1344 B contiguous writes per partition.
                rpp = CH * ntile
                dsty = y[row0:row0 + nrows, :].rearrange(
                    "(t r) j -> t r j", t=T_FULL, r=rpp)
                srcy = out_sb[:, 0:CH * NO * ntile].rearrange(
                    "t (r j) -> t r j", r=rpp, j=NO)
                nc.sync.dma_start(dsty, srcy)
            else:
                for ti, w in enumerate(widths):
                    r0 = row0 + CH * sum(widths[:ti])
                    dsty = y[r0:r0 + CH * w, :].rearrange(
                        "(t c) j -> t c j", t=w, c=CH)
                    srcy = out_sb[0:w, CH * NO * ti:CH * NO * (ti + 1)].rearrange(
                        "t (c j) -> t c j", c=CH, j=NO)
                    nc.sync.dma_start(dsty, srcy)

    def emit_all():
        # build the BB descriptor list
        bbs = []
        n_full_bb = full_tiles // BB_TILES
        for b in range(n_full_bb):
            bbs.append((b * BB_TILES * CH * T_FULL, BB_TILES, T_FULL))
        leftover = full_tiles - n_full_bb * BB_TILES
        row0 = n_full_bb * BB_TILES * CH * T_FULL
        if tail_t > 0:
            bbs.append((row0, leftover + 1, tail_t))
        elif leftover:
            bbs.append((row0, leftover, T_FULL))
        # pair them across the two streams; the next group's loads+transposes
        # are emitted MID-body (PE/DVE idle there), so at each boundary the
        # PE only has the G-matmuls between M7 and the next tanh1 — G(A)
        # overlaps tanh7(B) and ScalarE stays gapless.
        groups = []
        for i in range(0, len(bbs), 2):
            group = [(0, bbs[i])]
            if i + 1 < len(bbs):
                group.append((1, bbs[i + 1]))
            groups.append(group)
        if os.environ.get("K_NO_MIDTR"):
            # previous schedule: full head (loads+TR+G) at the boundary
            pending = None
            for group in groups:
                st = emit_loads(group)
                emit_tr(group, st)
                emit_g(group, st)
                if pending is not None:
                    emit_final(*pending)
                emit_body(group, st)
                pending = (group, st)
            if pending is not None:
                emit_final(*pending)
            return
        sts = {}
        pending = None
        for gi, group in enumerate(groups):
            if gi == 0:
                sts[0] = emit_loads(group)
                emit_tr(group, sts[0])
            emit_g(group, sts[gi])
            if pending is not None:
                emit_final(*pending)
            nxt = groups[gi + 1] if gi + 1 < len(groups) else None
            mid = None
            if nxt is not None:
                sts[gi + 1] = emit_loads(nxt)
                nxt_st = sts[gi + 1]
                mid = lambda g=nxt, s=nxt_st: emit_tr(g, s)
            emit_body(group, sts[gi], mid=mid)
            pending = (group, sts[gi])
        if pending is not None:
            emit_final(*pending)

    if nrep == 1:
        emit_all()
    else:
        with tc.For_i(0, nrep, 1):
            emit_all()


_CACHED = {}
PROFILE = False  # set True (e.g. from test.py) to capture an NTFF trace
LAST_RESULTS = None  # BassKernelResults of the most recent kernel() call


def _build_program(rows, nrep=1):
    nc = bacc.Bacc("TRN2", target_bir_lowering=False, debug=False,
                   num_devices=N_CORES)
    x = nc.dram_tensor("x", [rows, NF], F32, kind="ExternalInput").ap()
    y = nc.dram_tensor("y", [rows, NO], F32, kind="ExternalOutput").ap()
    gw = nc.dram_tensor("gw", [CH * NF, 128],
                        BF16 if S_DT_KIND == "bf16" else
                        (FP16 if S_DT_KIND == "fp16" else F32),
                        kind="ExternalInput").ap()
    wdt = {"bf16": BF16, "fp16": FP16}.get(S_DT_KIND, F32)
    mw = nc.dram_tensor("mw", [128, 128], wdt, kind="ExternalInput").ap()
    fw = nc.dram_tensor("fw", [128, CH * NO], wdt, kind="ExternalInput").ap()
    idm = nc.dram_tensor("idm", [128, 128], F32, kind="ExternalInput").ap()
    bvec = nc.dram_tensor("bvec", [128, 3], F32, kind="ExternalInput").ap()
    with tile.TileContext(nc) as tc, ExitStack() as ctx:
        build_tile_kernel(ctx, tc, x, y, gw, mw, fw, idm, bvec, rows, nrep=nrep)
    nc.compile()
    return nc


def kernel(x, Wj, bj, Wm, bm, Wih_j, Whh_j, bih_j, bhh_j,
           Wih_m, Whh_m, bih_m, bhh_m, Wact, bact):
    x = np.ascontiguousarray(np.asarray(x, dtype=np.float32))
    assert x.shape == (B_TOTAL, NF), x.shape
    G_lhsT, M_lhsT, F_rhs, bias, identity = build_host_constants(
        np.asarray(Wj, np.float32), np.asarray(bj, np.float32),
        np.asarray(Wm, np.float32), np.asarray(bm, np.float32),
        np.asarray(Wih_j, np.float32), np.asarray(Whh_j, np.float32),
        np.asarray(bih_j, np.float32), np.asarray(bhh_j, np.float32),
        np.asarray(Wih_m, np.float32), np.asarray(Whh_m, np.float32),
        np.asarray(bih_m, np.float32), np.asarray(bhh_m, np.float32),
        np.asarray(Wact, np.float32), np.asarray(bact, np.float32))

    if "nc" not in _CACHED:
        _CACHED["nc"] = _build_program(R)
    nc = _CACHED["nc"]

    if S_DT_KIND == "bf16":
        import ml_dtypes
        G_lhsT = G_lhsT.astype(ml_dtypes.bfloat16)
        M_lhsT = M_lhsT.astype(ml_dtypes.bfloat16)
        F_rhs = F_rhs.astype(ml_dtypes.bfloat16)
    elif S_DT_KIND == "fp16":
        G_lhsT = G_lhsT.astype(np.float16)
        M_lhsT = M_lhsT.astype(np.float16)
        F_rhs = F_rhs.astype(np.float16)
    in_maps = []
    for i in range(N_CORES):
        in_maps.append({
            "x": x[i * R:(i + 1) * R],
            "gw": G_lhsT, "mw": M_lhsT, "fw": F_rhs,
            "idm": identity, "bvec": bias,
        })
    res = run_bass_kernel_spmd(nc, in_maps, list(range(N_CORES)), trace=PROFILE)
    global LAST_RESULTS
    LAST_RESULTS = res
    out = np.concatenate([res.results[i]["y"] for i in range(N_CORES)], axis=0)
    return out



# revision 29
# speedup vs baseline: 1.2520x; 1.2520x over previous
"""Trainium2 Bass kernel for the AggregPolicy GNN message-passing model.

Math: the reference network is, per batch row x (18 features):
    s0 = E @ x_feats + d          (state s = [h_j[0..6] (7*4), h_m (4)] = 32 dims)
    s  = tanh(M @ s + c)          x 7   (chain-graph message passing folded into
                                         one dense 32x32 matrix M)
    out = F @ s + bact            (7 outputs)
The first iteration is fused with the init affine: s1 = tanh(G @ x + g) with
G = M @ E, g = M @ d + c.  All matrices are precomputed on the host from the
tiny model weights.

Layout on chip (per NeuronCore, pure data parallel over 8 cores):
  - 4 batch "chunks" x 32 state dims are stacked on the 128 SBUF partitions
    (block-diagonal G/M/F), batch runs along the free dimension.
  - batch rows are mapped so each SBUF partition of an input tile holds
    CH*ntile *consecutive* DRAM rows (>=512B contiguous reads/writes).
  - Pre-activations live in PSUM as fp16: 3072 columns fit in 3 banks per
    stream, so one ScalarE tanh instruction covers a whole big-batch (the
    ScalarE ACT stream is the bottleneck engine; fewer/longer ACTs win).
  - Two interleaved big-batch streams keep ScalarE gapless while the PE runs
    the other stream's matmuls; next group's transposes are emitted one
    fill per iteration gap so the in-order PE queue never carries a long
    transpose chain at once.
"""

import os

os.environ.setdefault("MYCRO_LOCAL_CACHE", "1")

from contextlib import ExitStack

import numpy as np

import concourse.bacc as bacc
import concourse.tile as tile
from concourse import mybir
from concourse.bass_utils import run_bass_kernel_spmd

F32 = mybir.dt.float32
BF16 = mybir.dt.bfloat16
FP16 = mybir.dt.float16
F32R = mybir.dt.float32r
S_DT_KIND = os.environ.get("K_SDT", "fp16")
SDT = {"bf16": BF16}.get(S_DT_KIND, FP16)

N_CORES = 8
B_TOTAL = 2_000_000
R = B_TOTAL // N_CORES  # 250000 rows per core
NF = 18  # input features
NS = 32  # state dims
NO = 7  # outputs
CH = 4  # batch chunks stacked on partitions (4*32 = 128)
T_FULL = 128  # batch columns per tile (rows per tile = 4*T_FULL = 512)
BB_TILES = 12  # tiles per big-batch (ACT op free dim = 128*BB_TILES = 1536)
CAP = T_FULL * BB_TILES
FILL_TILES = 4  # tiles per transpose fill (xt = [72, 512] f32r = 1 PSUM bank)
MM_W = 512  # matmul moving-operand slice (f32 out, 1 PSUM bank)

FULL_TILES = R // (CH * T_FULL)  # 488
TAIL_ROWS = R - FULL_TILES * CH * T_FULL  # 144
TAIL_T = TAIL_ROWS // CH  # 36
assert TAIL_T * CH == TAIL_ROWS

# Near-minimax odd polynomial tanh(z) ~= z*P(z^2) on |z|<=POLY_C, used by the
# VectorE to compute the LAST iteration's tanh for columns [ACT_COLS:ncols]
# (its error does not compound through further iterations, only through the
# tiny fc_act matrix; measured end-to-end rel err 5.7e-3 incl fp16 effects).
POLY_C = 3.5
POLY_A = (0.9798951004544019, -0.2637588070079745, 0.0562864824682624,
          -0.006989472172053205, 0.00044542309550251347,
          -1.1213419744181764e-05)


def build_host_constants(Wj, bj, Wm, bm, Wih_j, Whh_j, bih_j, bhh_j,
                         Wih_m, Whh_m, bih_m, bhh_m, Wact, bact):
    """Fold the model into (G_lhsT [72,128], M_lhsT [128,128], F_rhs [128,28],
    bias [128,3]) in the block-diagonal on-chip layouts."""
    H = 4
    M = np.zeros((NS, NS), np.float32)
    c = np.zeros((NS,), np.float32)
    for n in range(7):
        r = slice(4 * n, 4 * n + 4)
        if n == 0:
            M[r, 28:32] += Wih_j  # prev neighbor of node 0 is h_m
        else:
            M[r, 4 * (n - 1):4 * n] += Wih_j
        if n < 6:
            M[r, 4 * (n + 1):4 * (n + 2)] += Wih_j
        M[r, 4 * n:4 * n + 4] += Whh_j
        c[r] = bih_j + bhh_j
    M[28:32, 0:4] = Wih_m
    M[28:32, 28:32] = Whh_m
    c[28:32] = bih_m + bhh_m

    E = np.zeros((NS, NF), np.float32)
    d = np.zeros((NS,), np.float32)
    for n in range(7):
        for h in range(H):
            E[4 * n + h, 4 + n] = Wj[h, 0]
            E[4 * n + h, 11 + n] = Wj[h, 1]
        d[4 * n:4 * n + 4] = bj
    E[28:32, 0:4] = Wm
    d[28:32] = bm

    F = np.zeros((NO, NS), np.float32)
    for n in range(7):
        F[n, 4 * n:4 * n + 4] = Wact[0]

    G = (M @ E).astype(np.float32)
    g = (M @ d + c).astype(np.float32)

    # Block-diagonal device layouts.
    G_lhsT = np.zeros((CH * NF, 128), np.float32)
    for cc in range(CH):
        G_lhsT[NF * cc:NF * (cc + 1), NS * cc:NS * (cc + 1)] = G.T
    M_lhsT = np.zeros((128, 128), np.float32)
    for cc in range(CH):
        M_lhsT[NS * cc:NS * (cc + 1), NS * cc:NS * (cc + 1)] = M.T
    F_rhs = np.zeros((128, CH * NO), np.float32)
    for cc in range(CH):
        F_rhs[NS * cc:NS * (cc + 1), NO * cc:NO * (cc + 1)] = F.T
    # Per-partition bias vectors: col 0 = g (first iter), col 1 = c, col 2 = bact
    bias = np.zeros((128, 3), np.float32)
    bias[:, 0] = np.tile(g, CH)
    bias[:, 1] = np.tile(c, CH)
    bias[:, 2] = float(bact[0])
    identity = np.eye(128, dtype=np.float32)
    return G_lhsT, M_lhsT, F_rhs, bias, identity


def build_tile_kernel(ctx, tc, x, y, gw, mw, fw, idm, bvec, rows, nrep=1):
    """Emit the Tile program. x:[rows,18], y:[rows,7] DRAM APs; consts in DRAM.

    nrep > 1 wraps the whole body in a hardware For_i loop that recomputes the
    same outputs nrep times — used only for wall-clock benchmarking."""
    nc = tc.nc
    full_tiles = rows // (CH * T_FULL)
    tail_rows = rows - full_tiles * CH * T_FULL
    tail_t = tail_rows // CH
    assert tail_t * CH == tail_rows

    consts = ctx.enter_context(tc.tile_pool(name="consts", bufs=1))
    # per-stream SBUF pools
    xsb_p = [ctx.enter_context(tc.tile_pool(name=f"xsb{s}", bufs=2)) for s in (0, 1)]
    xf_p = [ctx.enter_context(tc.tile_pool(name=f"xf{s}", bufs=2)) for s in (0, 1)]
    s_p = [ctx.enter_context(tc.tile_pool(name=f"sp{s}", bufs=3)) for s in (0, 1)]
    osb_p = [ctx.enter_context(tc.tile_pool(name=f"osb{s}", bufs=4)) for s in (0, 1)]
    dv_p = [ctx.enter_context(tc.tile_pool(name=f"dv{s}", bufs=1)) for s in (0, 1)]
    # PSUM: 3 f32 banks of pre-activations per stream + 1 bank of transpose
    # staging (xt) + 1 bank for the final matmul output (outps). xt and outps
    # are SEPARATE pools so a fill's transpose (in-order PE queue!) never
    # waits on an outps bias-add that sits behind a long DVE poly chain.
    pre_p = [ctx.enter_context(tc.tile_pool(name=f"pre{s}", bufs=1, space="PSUM"))
             for s in (0, 1)]
    xt_p = ctx.enter_context(tc.tile_pool(name="xtp", bufs=1, space="PSUM"))
    ops_p = ctx.enter_context(tc.tile_pool(name="opsp", bufs=1, space="PSUM"))

    TANH = mybir.ActivationFunctionType.Tanh
    OP = mybir.AluOpType
    # ScalarE handles columns [0:ACT_COLS] of the last iteration; VectorE
    # computes the polynomial tanh for [ACT_COLS:ncols]. "all" disables.
    # Default "all": the last-iteration VectorE polynomial path (set e.g.
    # K_ACT_COLS=512 or K_ASYM=0 to enable it) measured no faster than the
    # pure-ScalarE schedule on hw, so it ships disabled; ScalarE tanh at
    # 1 elem/lane/cycle is the binding engine either way.
    _ac = os.environ.get("K_ACT_COLS", "all")
    ACT_COLS = 10 ** 9 if _ac == "all" else int(_ac)
    # asym mode: stream 0's last-iteration tanh goes entirely to the VectorE
    # (one long chain), stream 1 stays entirely on ScalarE (its long ACT covers
    # the pair-boundary PE work). K_ASYM=<cols of stream1 tail on DVE>.
    ASYM = os.environ.get("K_ASYM")
    ASYM_B_TAIL = int(ASYM) if ASYM else None
    ASYM_HEAD = int(os.environ.get("K_ASYM_HEAD", "256"))
    # benchmark-only ablation knobs (default = full correct kernel)
    N_TANH_ITERS = int(os.environ.get("K_ITERS", "7"))
    SKIP_F = bool(os.environ.get("K_SKIP_F"))
    SKIP_TR = bool(os.environ.get("K_SKIP_TR"))
    SKIP_DMA_IN = bool(os.environ.get("K_SKIP_DMA_IN"))
    SKIP_DMA_OUT = bool(os.environ.get("K_SKIP_DMA_OUT"))
    x = x.bitcast(F32R)
    idm = idm.bitcast(F32R)

    g_sb = consts.tile([CH * NF, 128], SDT, name="g_sb")
    nc.sync.dma_start(g_sb[:], gw)
    m_sb = consts.tile([128, 128], SDT, name="m_sb")
    nc.sync.dma_start(m_sb[:], mw)
    f_sb = consts.tile([128, CH * NO], SDT, name="f_sb")
    nc.sync.dma_start(f_sb[:], fw)
    id_sb = consts.tile([128, 128], F32R, name="id_sb")
    nc.sync.dma_start(id_sb[:], idm)
    b_sb = consts.tile([128, 3], F32, name="b_sb")
    nc.sync.dma_start(b_sb[:], bvec)

    def load_x(sl, bb):
        row0, ntile, t_last = bb
        widths = [T_FULL] * (ntile - 1) + [t_last]
        nrows = CH * sum(widths)
        dense = t_last == T_FULL
        x_sb = xsb_p[sl].tile([128, NF * CH * BB_TILES], F32R,
                              name=f"x_sb{sl}", tag="x_sb")
        if SKIP_DMA_IN:
            nc.vector.memset(x_sb[:, 0:2].bitcast(F32), 0.5)
        elif dense:
            # partition t holds CH*ntile consecutive DRAM rows -> one
            # contiguous >=512B read per partition.
            rpp = CH * ntile
            srcx = x[row0:row0 + nrows, :].rearrange(
                "(t r) f -> t r f", t=T_FULL, r=rpp)
            dstx = x_sb[:, 0:NF * CH * ntile].rearrange(
                "t (r f) -> t r f", r=rpp, f=NF)
            nc.sync.dma_start(dstx, srcx)
        else:
            for ti, w in enumerate(widths):
                r0 = row0 + CH * sum(widths[:ti])
                srcx = x[r0:r0 + CH * w, :].rearrange("(t c) f -> t c f", t=w, c=CH)
                dstx = x_sb[0:w, NF * CH * ti:NF * CH * (ti + 1)].rearrange(
                    "t (c f) -> t c f", c=CH, f=NF)
                nc.sync.dma_start(dstx, srcx)
        return x_sb

    def make_fills(sl, bb, st):
        """Return a list of fill closures; each transposes FILL_TILES tiles of
        x into the feature-major xfeat tile via a 1-bank PSUM staging tile."""
        row0, ntile, t_last = bb
        widths = [T_FULL] * (ntile - 1) + [t_last]
        xfeat = xf_p[sl].tile([CH * NF, CAP], SDT, name=f"xfeat{sl}", tag="xfeat")
        st["xfeat"] = xfeat
        if SKIP_TR:
            return []
        fills = []
        for f0 in range(0, ntile, FILL_TILES):
            gcnt = min(FILL_TILES, ntile - f0)

            def fill(f0=f0, gcnt=gcnt):
                x_sb = st["x_sb"]
                xt = xt_p.tile([CH * NF, FILL_TILES * T_FULL], F32R,
                               name="xt", tag="xt")
                for i in range(gcnt):
                    ti = f0 + i
                    w = widths[ti]
                    nc.tensor.transpose(
                        out=xt[0:CH * NF, T_FULL * i:T_FULL * i + w],
                        in_=x_sb[0:w, NF * CH * ti:NF * CH * (ti + 1)],
                        identity=id_sb[0:w, 0:w],
                    )
                if widths[f0 + gcnt - 1] == T_FULL:
                    nc.vector.tensor_copy(
                        xfeat[0:CH * NF, T_FULL * f0:T_FULL * (f0 + gcnt)],
                        xt[0:CH * NF, 0:T_FULL * gcnt])
                else:
                    for i in range(gcnt):
                        ti = f0 + i
                        w = widths[ti]
                        nc.vector.tensor_copy(
                            xfeat[0:CH * NF, T_FULL * ti:T_FULL * ti + w],
                            xt[0:CH * NF, T_FULL * i:T_FULL * i + w])

            fills.append(fill)
        return fills

    def mm_sliced(pre, lhsT, rhs_tile, rhs_parts, width):
        for q0 in range(0, width, MM_W):
            w = min(MM_W, width - q0)
            nc.tensor.matmul(
                pre[:, q0:q0 + w],
                lhsT=lhsT,
                rhs=rhs_tile[0:rhs_parts, q0:q0 + w],
                start=True, stop=True)

    def emit_loads(bbs):
        st = {}
        for sl, bb in bbs:
            st[sl] = {"bb": bb}
            st[sl]["ncols"] = T_FULL * (bb[1] - 1) + bb[2]
            st[sl]["x_sb"] = load_x(sl, bb)
        return st

    def collect_fills(bbs, st):
        fills = {sl: make_fills(sl, bb, st[sl]) for sl, bb in bbs}
        # interleave streams: A0, B0, A1, B1, ...
        out = []
        i = 0
        while True:
            got = False
            for sl, bb in bbs:
                if i < len(fills[sl]):
                    out.append(fills[sl][i])
                    got = True
            if not got:
                break
            i += 1
        return out

    def emit_g(bbs, st):
        """G-matmul: in the in-order PE stream this lands right after the
        previous body's M7, so G(A) overlaps tanh7(B) and tanh1 of this
        group starts with only the pre-tile WAR on tanh7 to wait for."""
        for sl, bb in bbs:
            pre = pre_p[sl].tile([128, CAP], F32, name=f"pre{sl}", tag="pre")
            st[sl]["pre"] = pre
            mm_sliced(pre, g_sb[:], st[sl]["xfeat"], CH * NF, st[sl]["ncols"])

    def poly_tanh_dve(sl, st, c0, c1):
        """VectorE: s[:, c0:c1] = polytanh(pre[:, c0:c1] + c) for the last
        iteration. pre is raw (the +c bias is normally applied inside the
        ACT's free affine), so fold it into the clamp chain."""
        pre = st["pre"]
        s_t = st["s"]
        d = c1 - c0
        a0, a1, a2, a3, a4, a5 = POLY_A

        def scr(tag):
            return dv_p[sl].tile([128, CAP], SDT,
                                 name=f"{tag}{sl}", tag=tag)[:, 0:d]

        zc, u, u2, u4, tA, tB, tC, t0, t1 = (
            scr(t) for t in ("zc", "u", "u2", "u4", "tA", "tB", "tC", "t0", "t1"))
        src = pre[:, c0:c1]
        # zc = min(z + c, POLY_C); then max with -POLY_C
        nc.vector.tensor_scalar(zc, src, b_sb[:, 1:2], POLY_C, OP.add, OP.min)
        nc.vector.tensor_scalar_max(zc, zc, -POLY_C)
        nc.vector.tensor_mul(u, zc, zc)
        nc.vector.tensor_mul(u2, u, u)
        nc.vector.tensor_mul(u4, u2, u2)
        nc.vector.tensor_scalar(tA, u, a1, a0, OP.mult, OP.add)
        nc.vector.tensor_scalar(tB, u, a3, a2, OP.mult, OP.add)
        nc.vector.tensor_scalar(tC, u, a5, a4, OP.mult, OP.add)
        nc.vector.tensor_mul(t0, u2, tB)
        nc.vector.tensor_mul(t1, u4, tC)
        nc.vector.tensor_add(t0, t0, tA)
        nc.vector.tensor_add(t0, t0, t1)
        nc.vector.tensor_mul(s_t[:, c0:c1], zc, t0)

    def emit_body(bbs, st, fills=(), finals=()):
        """One fill (transpose batch for the NEXT group) is emitted per
        iteration gap so the in-order PE queue interleaves them with this
        group's matmuls instead of stalling on one long transpose chain.
        The PREVIOUS group's per-stream final (F matmul + bias + store) is
        emitted at iteration gaps 3/4 — late enough that the previous DVE
        poly chain has drained, so F's wait on s never blocks this body's
        M-matmuls in the in-order PE queue."""
        fills = list(fills)
        finals = list(finals)
        fi = 0
        fin = 0
        for sl, bb in bbs:
            s_t = s_p[sl].tile([128, CAP], SDT, name=f"s{sl}", tag="s")
            st[sl]["s"] = s_t
            nc.scalar.activation(s_t[:, 0:st[sl]["ncols"]],
                                 st[sl]["pre"][:, 0:st[sl]["ncols"]], TANH,
                                 bias=b_sb[:, 0:1], scale=1.0)
        if fi < len(fills):
            fills[fi]()
            fi += 1
        # iterations 2..7
        for it in range(N_TANH_ITERS - 1):
            last = it == N_TANH_ITERS - 2
            for sl, bb in bbs:
                mm_sliced(st[sl]["pre"], m_sb[:], st[sl]["s"], 128,
                          st[sl]["ncols"])
            for sl, bb in bbs:
                n = st[sl]["ncols"]
                if last and ASYM_B_TAIL is not None:
                    na = ASYM_HEAD if sl == 0 else max(128, n - ASYM_B_TAIL)
                elif last:
                    na = min(ACT_COLS, n)
                else:
                    na = n
                if na > 0:
                    nc.scalar.activation(st[sl]["s"][:, 0:na],
                                         st[sl]["pre"][:, 0:na], TANH,
                                         bias=b_sb[:, 1:2], scale=1.0)
                if last and na < n:
                    poly_tanh_dve(sl, st[sl], na, n)
            if fi < len(fills):
                fills[fi]()
                fi += 1
            if it >= 2 and fin < len(finals):
                finals[fin]()
                fin += 1
        while fi < len(fills):
            fills[fi]()
            fi += 1
        while fin < len(finals):
            finals[fin]()
            fin += 1

    def emit_final_stream(sl, bb, st_sl):
        if SKIP_F:
            return
        row0, ntile, t_last = bb
        widths = [T_FULL] * (ntile - 1) + [t_last]
        outps = ops_p.tile([128, CH * NO * BB_TILES], F32,
                           name="outps", tag="outps")
        for ti, w in enumerate(widths):
            nc.tensor.matmul(
                outps[0:w, CH * NO * ti:CH * NO * (ti + 1)],
                lhsT=st_sl["s"][:, T_FULL * ti:T_FULL * ti + w],
                rhs=f_sb[:],
                start=True, stop=True)
        if True:
            dense = t_last == T_FULL
            out_sb = osb_p[sl].tile([128, CH * NO * BB_TILES], F32,
                                    name=f"out_sb{sl}", tag="out_sb")
            if dense:
                nc.vector.tensor_scalar_add(
                    out_sb[:, 0:CH * NO * ntile],
                    outps[:, 0:CH * NO * ntile], b_sb[:, 2:3])
            else:
                for ti, w in enumerate(widths):
                    nc.vector.tensor_scalar_add(
                        out_sb[0:w, CH * NO * ti:CH * NO * (ti + 1)],
                        outps[0:w, CH * NO * ti:CH * NO * (ti + 1)],
                        b_sb[0:w, 2:3])
            nrows = CH * sum(widths)
            if SKIP_DMA_OUT:
                return
            if dense:
                rpp = CH * ntile
                dsty = y[row0:row0 + nrows, :].rearrange(
                    "(t r) j -> t r j", t=T_FULL, r=rpp)
                srcy = out_sb[:, 0:CH * NO * ntile].rearrange(
                    "t (r j) -> t r j", r=rpp, j=NO)
                nc.sync.dma_start(dsty, srcy)
            else:
                for ti, w in enumerate(widths):
                    r0 = row0 + CH * sum(widths[:ti])
                    dsty = y[r0:r0 + CH * w, :].rearrange(
                        "(t c) j -> t c j", t=w, c=CH)
                    srcy = out_sb[0:w, CH * NO * ti:CH * NO * (ti + 1)].rearrange(
                        "t (c j) -> t c j", c=CH, j=NO)
                    nc.sync.dma_start(dsty, srcy)

    def emit_all():
        # build the BB descriptor list: (row0, ntile, t_last). Balance sizes:
        # an even number of BBs (2-stream pairs), each as close to BB_TILES
        # as possible, the tail tile folded into the last BB (<= CAP cols).
        n_bb = -(-full_tiles // BB_TILES)  # ceil
        has_tail = tail_t > 0
        if has_tail and full_tiles + 1 > n_bb * BB_TILES:
            n_bb += 1
        if n_bb % 2:
            n_bb += 1
        base = full_tiles // n_bb
        extra = full_tiles - base * n_bb  # this many BBs get base+1 tiles
        bbs = []
        row0 = 0
        for b in range(n_bb):
            ntile = base + (1 if b < extra else 0)
            tl = T_FULL
            if b == n_bb - 1 and has_tail:
                ntile += 1
                tl = tail_t
            assert T_FULL * (ntile - 1) + tl <= CAP
            bbs.append((row0, ntile, tl))
            row0 += CH * (T_FULL * (ntile - 1) + tl)
        groups = []
        for i in range(0, len(bbs), 2):
            group = [(0, bbs[i])]
            if i + 1 < len(bbs):
                group.append((1, bbs[i + 1]))
            groups.append(group)

        sts = {}
        pending = None
        for gi, group in enumerate(groups):
            if gi == 0:
                sts[0] = emit_loads(group)
                for f in collect_fills(group, sts[0]):
                    f()
            emit_g(group, sts[gi])
            finals = ()
            if pending is not None:
                pgroup, pst = pending
                finals = [
                    (lambda sl=sl, bb=bb, s=pst[sl]: emit_final_stream(sl, bb, s))
                    for sl, bb in pgroup]
            nxt = groups[gi + 1] if gi + 1 < len(groups) else None
            fills = ()
            if nxt is not None:
                sts[gi + 1] = emit_loads(nxt)
                fills = collect_fills(nxt, sts[gi + 1])
            emit_body(group, sts[gi], fills=fills, finals=finals)
            pending = (group, sts[gi])
        if pending is not None:
            for sl, bb in pending[0]:
                emit_final_stream(sl, bb, pending[1][sl])

    if nrep == 1:
        emit_all()
    else:
        with tc.For_i(0, nrep, 1):
            emit_all()


_CACHED = {}
PROFILE = False
LAST_RESULTS = None


def _build_program(rows, nrep=1):
    nc = bacc.Bacc("TRN2", target_bir_lowering=False, debug=False,
                   num_devices=N_CORES)
    x = nc.dram_tensor("x", [rows, NF], F32, kind="ExternalInput").ap()
    y = nc.dram_tensor("y", [rows, NO], F32, kind="ExternalOutput").ap()
    wdt = BF16 if S_DT_KIND == "bf16" else FP16
    gw = nc.dram_tensor("gw", [CH * NF, 128], wdt, kind="ExternalInput").ap()
    mw = nc.dram_tensor("mw", [128, 128], wdt, kind="ExternalInput").ap()
    fw = nc.dram_tensor("fw", [128, CH * NO], wdt, kind="ExternalInput").ap()
    idm = nc.dram_tensor("idm", [128, 128], F32, kind="ExternalInput").ap()
    bvec = nc.dram_tensor("bvec", [128, 3], F32, kind="ExternalInput").ap()
    with tile.TileContext(nc) as tc, ExitStack() as ctx:
        build_tile_kernel(ctx, tc, x, y, gw, mw, fw, idm, bvec, rows, nrep=nrep)
    nc.compile()
    return nc


def kernel(x, Wj, bj, Wm, bm, Wih_j, Whh_j, bih_j, bhh_j,
           Wih_m, Whh_m, bih_m, bhh_m, Wact, bact):
    x = np.ascontiguousarray(np.asarray(x, dtype=np.float32))
    assert x.shape == (B_TOTAL, NF), x.shape
    G_lhsT, M_lhsT, F_rhs, bias, identity = build_host_constants(
        np.asarray(Wj, np.float32), np.asarray(bj, np.float32),
        np.asarray(Wm, np.float32), np.asarray(bm, np.float32),
        np.asarray(Wih_j, np.float32), np.asarray(Whh_j, np.float32),
        np.asarray(bih_j, np.float32), np.asarray(bhh_j, np.float32),
        np.asarray(Wih_m, np.float32), np.asarray(Whh_m, np.float32),
        np.asarray(bih_m, np.float32), np.asarray(bhh_m, np.float32),
        np.asarray(Wact, np.float32), np.asarray(bact, np.float32))

    if "nc" not in _CACHED:
        _CACHED["nc"] = _build_program(R)
    nc = _CACHED["nc"]

    if S_DT_KIND == "bf16":
        import ml_dtypes
        wnp = ml_dtypes.bfloat16
    else:
        wnp = np.float16
    G_lhsT = G_lhsT.astype(wnp)
    M_lhsT = M_lhsT.astype(wnp)
    F_rhs = F_rhs.astype(wnp)
    in_maps = []
    for i in range(N_CORES):
        in_maps.append({
            "x": x[i * R:(i + 1) * R],
            "gw": G_lhsT, "mw": M_lhsT, "fw": F_rhs,
            "idm": identity, "bvec": bias,
        })
    res = run_bass_kernel_spmd(nc, in_maps, list(range(N_CORES)), trace=PROFILE)
    global LAST_RESULTS
    LAST_RESULTS = res
    out = np.concatenate([res.results[i]["y"] for i in range(N_CORES)], axis=0)
    return out
